# revision 10
# baseline (speedup 1.0000x reference)
"""GCNConv kernel for Trainium2, SPMD over 8 NeuronCores.

Math (matches the reference):
    row = [edge_index[0], arange(N)]; col = [edge_index[1], arange(N)]
    deg = bincount(row); dis = deg ** -0.5
    agg[c] = sum_{e: col_e == c} dis[row_e] * dis[c] * x[row_e]
    out = agg @ W.T + b

Distribution: edges are sorted by destination column; core k owns
destination columns [k*12500, (k+1)*12500).  Outputs are disjoint across
cores -> no collectives.

Host preprocessing builds, per core, the edge-expanded message stream
x_dis[row_e] (bf16, already scaled by the source-side dis factor) laid out
in [128-edge x 128-feat] tiles grouped by destination column block, plus an
exact fp8 0/1 one-hot S mapping each edge slot to its local destination
column.  (The per-edge expansion is done host-side: the batched device
gather primitive -- InstDMAGatherAnt -- corrupts index values >= 256 on
this toolchain (empirically the idx stream is rounded like an 8-bit-
mantissa float), and the working [128,1]-offset indirect-DMA fallback
costs a measured 1.44 us per 128 edges = ~2.7 ms/core, 8x over the memory
roofline.  Multi-offset indirect DMA ([128,K>=2]) gathers wrong rows for
partitions >= 64 -- all verified on hardware.)

Per-core device program (the scatter/segment-sum and the linear):
  - for each group of GRP=4 column blocks (128 cols each):
      * stream the group's message tiles into SBUF (contiguous DMA)
      * per block: K matmuls  psum[d, c] += G_tile^T @ S_tile  accumulate
        the block's transposed segment sums in PSUM
      * drain PSUM with a fused multiply by dis[col] (fp32, broadcast
        tile), fp32 matmul with W^T, per-partition bias add, store the
        [dout, col] tile to the transposed output
  - host transposes each core's [128, 12500] slab into the final output.
"""

import numpy as np
import ml_dtypes

BF16 = ml_dtypes.bfloat16
FP8 = ml_dtypes.float8_e4m3

N_NODES = 100000
D = 128
N_CORES = 8
BLK = 128
GRP = 4  # column blocks per DMA group


def _preprocess(x, edge_index, W, b, n_cores=N_CORES, grp=GRP):
    """Host-side index preprocessing and input sharding."""
    n = x.shape[0]
    d = x.shape[1]
    assert n % n_cores == 0
    cpc = n // n_cores  # destination columns per core
    nblk = -(-cpc // BLK)  # column blocks per core
    pcols = nblk * BLK
    ng = -(-nblk // grp)  # groups per core

    idt = edge_index.dtype
    loop = np.arange(n, dtype=idt)
    row = np.concatenate([np.asarray(edge_index[0]), loop])
    col = np.concatenate([np.asarray(edge_index[1]), loop])

    deg = np.bincount(row, minlength=n)
    dis = (deg.astype(np.float64) ** -0.5).astype(np.float32)

    x_dis = (np.asarray(x) * dis[:, None]).astype(BF16)

    order = np.argsort(col, kind="stable")
    rs = row[order].astype(np.int64)
    cs = col[order].astype(np.int64)

    core = cs // cpc
    lb = (cs - core * cpc) // BLK  # block within core
    cloc = (cs - core * cpc) % BLK  # column within block

    key = core * nblk + lb
    counts = np.bincount(key, minlength=n_cores * nblk)
    starts = np.concatenate([[0], np.cumsum(counts)[:-1]])
    rank = np.arange(len(cs)) - starts[key]
    kmax = int(-(-counts.max() // 128))

    g = lb // grp
    bl = lb % grp
    kt = rank // 128
    p = rank % 128
    w_idx = grp * kmax

    # edge-expanded message stream: [core][g][p][bl*kmax+kt][128 feat]
    xe = np.zeros((n_cores, ng, 128, w_idx, d), BF16)
    flat_tile = ((core * ng + g) * 128 + p) * w_idx + bl * kmax + kt
    xe.reshape(-1, d)[flat_tile] = x_dis[rs]

    s_all = np.zeros((n_cores, ng, 128, w_idx * 128), FP8)
    flat_s = ((core * ng + g) * 128 + p) * (w_idx * 128) + (bl * kmax + kt) * 128 + cloc
    s_all.reshape(-1)[flat_s] = FP8(1.0)

    disb_all = np.zeros((n_cores, 128, pcols), np.float32)
    for k in range(n_cores):
        dc = np.zeros(pcols, np.float32)
        dc[:cpc] = dis[k * cpc : (k + 1) * cpc]
        disb_all[k] = dc[None, :]

    wt = np.ascontiguousarray(np.asarray(W).T.astype(np.float32))
    bias = np.asarray(b).astype(np.float32).reshape(d, 1)

    in_maps = []
    for k in range(n_cores):
        in_maps.append(
            {
                "xe": xe[k].reshape(ng, 128, w_idx * d),
                "soh": s_all[k],
                "wt": wt,
                "bias": bias,
                "disb": disb_all[k],
            }
        )
    meta = dict(n=n, d=d, cpc=cpc, nblk=nblk, pcols=pcols, ng=ng, kmax=kmax, grp=grp)
    return in_maps, meta


def _build_program(meta):
    import concourse.bacc as bacc
    import concourse.tile as tile
    from concourse import mybir

    d = meta["d"]
    ng = meta["ng"]
    grp = meta["grp"]
    kmax = meta["kmax"]
    pcols = meta["pcols"]
    nblk = meta["nblk"]
    w_idx = grp * kmax

    f32 = mybir.dt.float32
    bf16 = mybir.dt.bfloat16
    fp8 = mybir.dt.float8e4

    nc = bacc.Bacc("TRN2", target_bir_lowering=False, debug=False)
    xe_t = nc.declare_dram_parameter("xe", [ng, 128, w_idx * d], bf16, isOutput=False)
    s_t = nc.declare_dram_parameter("soh", [ng, 128, w_idx * 128], fp8, isOutput=False)
    wt_t = nc.declare_dram_parameter("wt", [d, d], f32, isOutput=False)
    b_t = nc.declare_dram_parameter("bias", [d, 1], f32, isOutput=False)
    d_t = nc.declare_dram_parameter("disb", [128, pcols], f32, isOutput=False)
    o_t = nc.declare_dram_parameter("outT", [128, pcols], f32, isOutput=True)

    with tile.TileContext(nc) as tc:
        with (
            tc.tile_pool(name="const", bufs=1) as constp,
            tc.tile_pool(name="gather", bufs=2) as gatherp,
            tc.tile_pool(name="sohp", bufs=2) as sohp,
            tc.tile_pool(name="aggp", bufs=3) as aggp,
            tc.tile_pool(name="outp", bufs=3) as outp,
            tc.tile_pool(name="psA", bufs=4, space="PSUM") as psa,
            tc.tile_pool(name="psB", bufs=2, space="PSUM") as psb,
        ):
            wt_sb = constp.tile([d, d], f32)
            nc.sync.dma_start(out=wt_sb[:], in_=wt_t[:])
            b_sb = constp.tile([d, 1], f32)
            nc.sync.dma_start(out=b_sb[:], in_=b_t[:])
            disb_sb = constp.tile([128, pcols], f32)
            nc.sync.dma_start(out=disb_sb[:], in_=d_t[:])

            for gi in range(ng):
                ar = gatherp.tile([128, w_idx * d], bf16)
                nc.sync.dma_start(out=ar[:], in_=xe_t[gi])
                s_sb = sohp.tile([128, w_idx * 128], fp8)
                nc.sync.dma_start(out=s_sb[:], in_=s_t[gi])

                for bl in range(grp):
                    blk = gi * grp + bl
                    if blk >= nblk:
                        break
                    ps = psa.tile([128, 128], f32)
                    for kt in range(kmax):
                        off = (bl * kmax + kt) * 128
                        nc.tensor.matmul(
                            out=ps[:],
                            lhsT=ar[:, off : off + 128],
                            rhs=s_sb[:, off : off + 128],
                            start=(kt == 0),
                            stop=(kt == kmax - 1),
                        )
                    agg_sb = aggp.tile([128, 128], f32)
                    nc.vector.tensor_tensor(
                        out=agg_sb[:],
                        in0=ps[:],
                        in1=disb_sb[:, blk * 128 : (blk + 1) * 128],
                        op=mybir.AluOpType.mult,
                    )
                    ps2 = psb.tile([128, 128], f32)
                    nc.tensor.matmul(
                        out=ps2[:], lhsT=wt_sb[:], rhs=agg_sb[:], start=True, stop=True
                    )
                    out_sb = outp.tile([128, 128], f32)
                    nc.vector.tensor_scalar_add(
                        out=out_sb[:], in0=ps2[:], scalar1=b_sb[:]
                    )
                    nc.sync.dma_start(
                        out=o_t[:, blk * 128 : (blk + 1) * 128], in_=out_sb[:]
                    )
    nc.compile()
    return nc


def _run(in_maps, meta, trace=False):
    from concourse.bass_utils import run_bass_kernel_spmd

    nc = _build_program(meta)
    n_cores = len(in_maps)
    res = run_bass_kernel_spmd(nc, in_maps, list(range(n_cores)), trace=trace)
    return res


def _assemble(results, meta, n_cores):
    cpc = meta["cpc"]
    out = np.empty((meta["n"], meta["d"]), np.float32)
    for k in range(n_cores):
        out[k * cpc : (k + 1) * cpc, :] = results[k]["outT"][:, :cpc].T
    return out


def kernel(x, edge_index, W, b):
    in_maps, meta = _preprocess(x, edge_index, W, b)
    res = _run(in_maps, meta, trace=False)
    return _assemble(res.results, meta, N_CORES)


# revision 11
# speedup vs baseline: 1.3933x; 1.3933x over previous
"""GCNConv kernel for Trainium2, SPMD over 8 NeuronCores.

Math (matches the reference):
    row = [edge_index[0], arange(N)]; col = [edge_index[1], arange(N)]
    deg = bincount(row); dis = deg ** -0.5
    agg[c] = sum_{e: col_e == c} dis[row_e] * dis[c] * x[row_e]
    out = agg @ W.T + b

Distribution: edges are sorted by destination column; core k owns
destination columns [k*12500, (k+1)*12500).  Outputs are disjoint across
cores -> no collectives.

Host preprocessing builds, per core, the edge-expanded message stream
x_dis[row_e] (bf16, already scaled by the source-side dis factor) laid out
in [128-edge x 128-feat] tiles grouped by destination column block, plus an
exact fp8 0/1 one-hot S mapping each edge slot to its local destination
column.  (The per-edge expansion is done host-side: the batched device
gather primitive -- InstDMAGatherAnt -- corrupts index values >= 256 on
this toolchain (empirically the idx stream is rounded like an 8-bit-
mantissa float), and the working [128,1]-offset indirect-DMA fallback
costs a measured 1.44 us per 128 edges = ~2.7 ms/core, 8x over the memory
roofline.  Multi-offset indirect DMA ([128,K>=2]) gathers wrong rows for
partitions >= 64 -- all verified on hardware.)

Per-core device program (the scatter/segment-sum and the linear):
  - for each group of GRP=4 column blocks (128 cols each):
      * stream the group's message tiles into SBUF (contiguous DMA)
      * per block: K matmuls  psum[d, c] += G_tile^T @ S_tile  accumulate
        the block's transposed segment sums in PSUM
      * drain PSUM with a fused multiply by dis[col] (fp32, broadcast
        tile), fp32 matmul with W^T, per-partition bias add, store the
        [dout, col] tile to the transposed output
  - host transposes each core's [128, 12500] slab into the final output.
"""

import numpy as np
import ml_dtypes

BF16 = ml_dtypes.bfloat16
FP8 = ml_dtypes.float8_e4m3

N_NODES = 100000
D = 128
N_CORES = 8
BLK = 128
GRP = 4  # column blocks per DMA group


def _preprocess(x, edge_index, W, b, n_cores=N_CORES, grp=GRP):
    """Host-side index preprocessing and input sharding."""
    n = x.shape[0]
    d = x.shape[1]
    assert n % n_cores == 0
    cpc = n // n_cores  # destination columns per core
    nblk = -(-cpc // BLK)  # column blocks per core
    pcols = nblk * BLK
    ng = -(-nblk // grp)  # groups per core

    idt = edge_index.dtype
    loop = np.arange(n, dtype=idt)
    row = np.concatenate([np.asarray(edge_index[0]), loop])
    col = np.concatenate([np.asarray(edge_index[1]), loop])

    deg = np.bincount(row, minlength=n)
    dis = (deg.astype(np.float64) ** -0.5).astype(np.float32)

    x_dis = (np.asarray(x) * dis[:, None]).astype(BF16)

    order = np.argsort(col, kind="stable")
    rs = row[order].astype(np.int64)
    cs = col[order].astype(np.int64)

    core = cs // cpc
    lb = (cs - core * cpc) // BLK  # block within core
    cloc = (cs - core * cpc) % BLK  # column within block

    key = core * nblk + lb
    counts = np.bincount(key, minlength=n_cores * nblk)
    starts = np.concatenate([[0], np.cumsum(counts)[:-1]])
    rank = np.arange(len(cs)) - starts[key]
    kmax = int(-(-counts.max() // 128))

    g = lb // grp
    bl = lb % grp
    kt = rank // 128
    p = rank % 128
    w_idx = grp * kmax

    # edge-expanded message stream: [core][g][p][bl*kmax+kt][128 feat]
    xe = np.zeros((n_cores, ng, 128, w_idx, d), BF16)
    flat_tile = ((core * ng + g) * 128 + p) * w_idx + bl * kmax + kt
    xe.reshape(-1, d)[flat_tile] = x_dis[rs]

    s_all = np.zeros((n_cores, ng, 128, w_idx * 128), FP8)
    flat_s = ((core * ng + g) * 128 + p) * (w_idx * 128) + (bl * kmax + kt) * 128 + cloc
    s_all.reshape(-1)[flat_s] = FP8(1.0)

    disb_all = np.zeros((n_cores, 128, pcols), np.float32)
    for k in range(n_cores):
        dc = np.zeros(pcols, np.float32)
        dc[:cpc] = dis[k * cpc : (k + 1) * cpc]
        disb_all[k] = dc[None, :]

    wt = np.ascontiguousarray(np.asarray(W).T.astype(np.float32))
    bias = np.asarray(b).astype(np.float32).reshape(d, 1)

    in_maps = []
    for k in range(n_cores):
        in_maps.append(
            {
                "xe": xe[k].reshape(ng, 128, w_idx * d),
                "soh": s_all[k],
                "wt": wt,
                "bias": bias,
                "disb": disb_all[k],
            }
        )
    meta = dict(n=n, d=d, cpc=cpc, nblk=nblk, pcols=pcols, ng=ng, kmax=kmax, grp=grp)
    return in_maps, meta


def _build_program(meta):
    import concourse.bacc as bacc
    import concourse.tile as tile
    from concourse import mybir

    d = meta["d"]
    ng = meta["ng"]
    grp = meta["grp"]
    kmax = meta["kmax"]
    pcols = meta["pcols"]
    nblk = meta["nblk"]
    w_idx = grp * kmax

    f32 = mybir.dt.float32
    bf16 = mybir.dt.bfloat16
    fp8 = mybir.dt.float8e4

    nc = bacc.Bacc("TRN2", target_bir_lowering=False, debug=False)
    xe_t = nc.declare_dram_parameter("xe", [ng, 128, w_idx * d], bf16, isOutput=False)
    s_t = nc.declare_dram_parameter("soh", [ng, 128, w_idx * 128], fp8, isOutput=False)
    wt_t = nc.declare_dram_parameter("wt", [d, d], f32, isOutput=False)
    b_t = nc.declare_dram_parameter("bias", [d, 1], f32, isOutput=False)
    d_t = nc.declare_dram_parameter("disb", [128, pcols], f32, isOutput=False)
    o_t = nc.declare_dram_parameter("outT", [128, pcols], f32, isOutput=True)

    with tile.TileContext(nc) as tc:
        with (
            tc.tile_pool(name="const", bufs=1) as constp,
            tc.tile_pool(name="gather", bufs=3) as gatherp,
            tc.tile_pool(name="sohp", bufs=3) as sohp,
            tc.tile_pool(name="aggp", bufs=3) as aggp,
            tc.tile_pool(name="outp", bufs=2) as outp,
            tc.tile_pool(name="psA", bufs=4, space="PSUM") as psa,
            tc.tile_pool(name="psB", bufs=2, space="PSUM") as psb,
        ):
            # constants go on the scalar (ACT) HWDGE queue so the sync queue
            # stays a pure xe/soh prefetch stream
            wt_sb = constp.tile([d, d], f32)
            nc.scalar.dma_start(out=wt_sb[:], in_=wt_t[:])
            b_sb = constp.tile([d, 1], f32)
            nc.scalar.dma_start(out=b_sb[:], in_=b_t[:])
            disb_sb = constp.tile([128, pcols], f32)
            nc.scalar.dma_start(out=disb_sb[:], in_=d_t[:])

            for gi in range(ng):
                ar = gatherp.tile([128, w_idx * d], bf16)
                nc.sync.dma_start(out=ar[:], in_=xe_t[gi])
                s_sb = sohp.tile([128, w_idx * 128], fp8)
                nc.sync.dma_start(out=s_sb[:], in_=s_t[gi])

                gblk = min(grp, nblk - gi * grp)  # blocks in this group
                out_g = outp.tile([128, grp * 128], f32, tag="outg")
                for bl in range(gblk):
                    blk = gi * grp + bl
                    ps = psa.tile([128, 128], f32)
                    for kt in range(kmax):
                        off = (bl * kmax + kt) * 128
                        nc.tensor.matmul(
                            out=ps[:],
                            lhsT=ar[:, off : off + 128],
                            rhs=s_sb[:, off : off + 128],
                            start=(kt == 0),
                            stop=(kt == kmax - 1),
                        )
                    agg_sb = aggp.tile([128, 128], f32)
                    nc.vector.tensor_tensor(
                        out=agg_sb[:],
                        in0=ps[:],
                        in1=disb_sb[:, blk * 128 : (blk + 1) * 128],
                        op=mybir.AluOpType.mult,
                    )
                    ps2 = psb.tile([128, 128], f32)
                    nc.tensor.matmul(
                        out=ps2[:], lhsT=wt_sb[:], rhs=agg_sb[:], start=True, stop=True
                    )
                    nc.vector.tensor_scalar_add(
                        out=out_g[:, bl * 128 : (bl + 1) * 128],
                        in0=ps2[:],
                        scalar1=b_sb[:],
                    )
                # one batched store per group, on the scalar HWDGE queue
                nc.scalar.dma_start(
                    out=o_t[:, gi * grp * 128 : gi * grp * 128 + gblk * 128],
                    in_=out_g[:, : gblk * 128],
                )
    nc.compile()
    return nc


def _run(in_maps, meta, trace=False):
    from concourse.bass_utils import run_bass_kernel_spmd

    nc = _build_program(meta)
    n_cores = len(in_maps)
    res = run_bass_kernel_spmd(nc, in_maps, list(range(n_cores)), trace=trace)
    return res


def _assemble(results, meta, n_cores):
    cpc = meta["cpc"]
    out = np.empty((meta["n"], meta["d"]), np.float32)
    for k in range(n_cores):
        out[k * cpc : (k + 1) * cpc, :] = results[k]["outT"][:, :cpc].T
    return out


def kernel(x, edge_index, W, b):
    in_maps, meta = _preprocess(x, edge_index, W, b)
    res = _run(in_maps, meta, trace=False)
    return _assemble(res.results, meta, N_CORES)
